# revision 29
# baseline (speedup 1.0000x reference)
"""Trainium2 Bass kernel for causal multi-head attention (8-core SPMD).

Problem: B=2, S=2048, H=2048, 16 heads (hd=128), RoPE, causal mask,
layer-index scaling (/4), additive pad mask (zeros by construction).

Sharding: core c handles batch b=c//4 and head-group g=c%4 (4 heads).
wq/wk/wv column-parallel, wo row-parallel; host sums the 4 partial
outputs per batch.

Per-core dataflow (all feature-on-partition, "transposed" layouts):
  qT/kT [d=128, S] = w.T-tile @ xT      (PSUM accum over H-chunks)
    Q/K projections run in fp8 e4m3 with DoubleRow perf mode (2 128-row
    contraction chunks per matmul, 2x PE throughput).  x and w are
    pre-scaled into fp8 range on host; the descale and the softmax
    scale are folded into the exp activation's scale argument.
  RoPE on qT/kT via head-dim permutation chosen so the rotation pair
    sits 16 partitions apart inside each 32-partition quadrant
    (stream_shuffle does the swap in one DVE op)
  scoresT [k,q] tiles = kT-tile.T @ qT-block ; exp on ACT -> PT (bf16)
  diagonal tiles masked multiplicatively post-exp
  row sums via all-ones stationary matmul (broadcast over partitions)
  OT [d, q] += v-tile.T @ PT ; normalized by reciprocal(sums)
  out_partial = OT.T @ woT  (PSUM accum over the 4 local head chunks)

V projection, scores, attV and the out projection run in bf16 (fp32
PSUM accumulation); softmax math in fp32.  Output partials ship bf16.
"""
import math
import os
import sys

import numpy as np

for _p in ("/opt/trn_rl_repo", "/root/.axon_site/_ro/trn_rl_repo"):
    if os.path.isdir(_p) and _p not in sys.path:
        sys.path.append(_p)

import ml_dtypes

S = 2048
H = 2048
NHEADS = 16
HD = 128
NH_LOC = 4          # heads per core
D_LOC = NH_LOC * HD  # 512
LAYER_INDEX = 3
SCALE = 1.0 / (math.sqrt(HD) * (LAYER_INDEX + 1))
N_CORES = 8
SB = 512            # S-block (matmul moving free dim)
HC = H // 128       # contraction chunks
NPAIR = HC // 2     # DoubleRow pair-chunks

# fp8 pre-scales: keep quantized values in e4m3's normal range
SW8 = 8.0           # weight scale (w sigma 0.022 -> 0.18)
SX8 = 1.0           # x scale (already unit variance)
DESCALE = 1.0 / (SW8 * SX8)

# head-dim permutation: RoPE pair (x1_j, x2_j) -> rows (qd*32 + j%16,
# qd*32 + 16 + j%16) with qd = j//16, so the swap is within-quadrant.
_P_NEW2OLD = np.zeros(HD, dtype=np.int64)
_J_OF_P = np.zeros(HD, dtype=np.int64)
_SIGN_OF_P = np.zeros(HD, dtype=np.float32)
for _p in range(HD):
    _qd, _r = _p // 32, _p % 32
    _j = _qd * 16 + (_r % 16)
    _P_NEW2OLD[_p] = 2 * _j + (1 if _r >= 16 else 0)
    _J_OF_P[_p] = _j
    _SIGN_OF_P[_p] = 1.0 if _r >= 16 else -1.0
_SHUF_MASK = [(i + 16) % 32 for i in range(32)]

_BF16 = ml_dtypes.bfloat16
_F8 = ml_dtypes.float8_e4m3
_NC_CACHE = {}


def _build_nc():
    import concourse.bacc as bacc
    import concourse.mybir as mybir
    import concourse.tile as tile

    f32 = mybir.dt.float32
    bf16 = mybir.dt.bfloat16
    f8 = mybir.dt.float8e4
    DR = mybir.MatmulPerfMode.DoubleRow
    Exp = mybir.ActivationFunctionType.Exp

    nc = bacc.Bacc("TRN2", target_bir_lowering=False, debug=False)

    # x and weight layouts are host-repacked "[p-outer, chunk-contiguous]"
    # so every DMA line is 2-4KB (512B lines throttle early bandwidth)
    xt8_d = nc.dram_tensor("xt8", [4 * 128, HC * SB], f8,
                           kind="ExternalInput")
    xtb_d = nc.dram_tensor("xtb", [4 * 128, HC * SB], bf16,
                           kind="ExternalInput")
    wq8_d = nc.dram_tensor("wq8", [128, HC * D_LOC], f8,
                           kind="ExternalInput")
    wk8_d = nc.dram_tensor("wk8", [128, HC * D_LOC], f8,
                           kind="ExternalInput")
    wvt_d = nc.dram_tensor("wvt", [128, HC * D_LOC], bf16,
                           kind="ExternalInput")
    wot_d = nc.dram_tensor("wot", [D_LOC, H], bf16, kind="ExternalInput")
    cos_d = nc.dram_tensor("cos_pm", [128, S], bf16, kind="ExternalInput")
    sin_d = nc.dram_tensor("sin_pm", [128, S], bf16, kind="ExternalInput")
    tri_d = nc.dram_tensor("tri", [128, 128], bf16, kind="ExternalInput")
    ones_d = nc.dram_tensor("ones", [128, 128], bf16, kind="ExternalInput")
    out_d = nc.dram_tensor("out_partial", [S, H], bf16, kind="ExternalOutput")

    n_sb = S // SB       # 4
    n_st = S // 128      # 16
    EXP_SCALE = SCALE * DESCALE * DESCALE

    with tile.TileContext(nc) as tc:
        with (
            tc.tile_pool(name="const", bufs=1) as const_pool,
            tc.tile_pool(name="qkv", bufs=1) as qkv_pool,
        ):
            cos_t = const_pool.tile([128, S], bf16, tag="cos")
            sin_t = const_pool.tile([128, S], bf16, tag="sin")
            tri_t = const_pool.tile([128, 128], bf16, tag="tri")
            ones_t = const_pool.tile([128, 128], bf16, tag="ones")

            qT = qkv_pool.tile([128, NH_LOC, S], bf16, tag="qT")
            kT = qkv_pool.tile([128, NH_LOC, S], bf16, tag="kT")
            v_t = qkv_pool.tile([128, n_st, D_LOC], bf16, tag="v")

            # ---------------- Phase A: projections + RoPE ----------------
            with (
                tc.tile_pool(name="w", bufs=1) as w_pool,
                tc.tile_pool(name="xtp8", bufs=2) as xt8_pool,
                tc.tile_pool(name="xtpb", bufs=4) as xtb_pool,
                tc.tile_pool(name="rope", bufs=2) as rope_pool,
                tc.tile_pool(name="psA", bufs=2, space="PSUM") as psA,
            ):
                wq_t = w_pool.tile([128, HC, D_LOC], f8, tag="wq")
                wk_t = w_pool.tile([128, HC, D_LOC], f8, tag="wk")
                wv_t = w_pool.tile([128, HC, D_LOC], bf16, tag="wv")

                # PE warmup: the HAM clock gate needs ~3.4us of sustained
                # matmul activity to lift the cold 1.2GHz throttle, and the
                # first DMA chunks only land ~9us in.  Run throwaway
                # matmuls on a memset tile so the real projections start
                # at full clock.
                warm_sb = rope_pool.tile([128, SB], bf16, tag="warm", bufs=1)
                nc.vector.memset(warm_sb[:], 0)
                warm_ps = psA.tile([128, SB], f32, tag="pqk", bufs=8,
                                   name="warm")
                for i in range(22):
                    nc.tensor.matmul(warm_ps[:], warm_sb[:, 0:128],
                                     warm_sb[:], start=(i == 0),
                                     stop=(i == 21))

                xt8_view = xt8_d[:, :].rearrange(
                    "(sb p) (hc f) -> sb p hc f", p=128, hc=HC)
                xtb_view = xtb_d[:, :].rearrange(
                    "(sb p) (hc f) -> sb p hc f", p=128, hc=HC)
                wq_view = wq8_d[:, :].rearrange("p (hc d) -> p hc d", hc=HC)
                wk_view = wk8_d[:, :].rearrange("p (hc d) -> p hc d", hc=HC)
                wv_view = wvt_d[:, :].rearrange("p (hc d) -> p hc d", hc=HC)

                # Early-DMA priority: the interleaved q+k passes need xt8 +
                # wq8 + wk8 chunks immediately — one critical stream per
                # queue so none is starved.  RoPE tables follow (needed at
                # ~+20us), the bulk xtb/wv loads last (v-passes run after
                # all q+k, ~+60us).
                chunks = [(0, 2), (2, 2), (4, 4), (8, 4), (12, 4)]
                xt8_blks = []
                for sb in range(n_sb):
                    blk = xt8_pool.tile([128, HC, SB], f8, tag="xt8",
                                        bufs=3)
                    for c0, w in (chunks if sb == 0 else [(0, 8), (8, 8)]):
                        csl = slice(c0, c0 + w)
                        nc.sync.dma_start(blk[:, csl, :],
                                          xt8_view[sb][:, csl, :])
                    xt8_blks.append(blk)
                for c0, w in chunks:
                    csl = slice(c0, c0 + w)
                    nc.scalar.dma_start(wq_t[:, csl, :], wq_view[:, csl, :])
                    nc.gpsimd.dma_start(wk_t[:, csl, :], wk_view[:, csl, :])
                nc.gpsimd.dma_start(sin_t[:], sin_d[:, :])
                nc.gpsimd.dma_start(cos_t[:], cos_d[:, :])
                nc.gpsimd.dma_start(tri_t[:], tri_d[:, :])
                nc.gpsimd.dma_start(ones_t[:], ones_d[:, :])
                xtb_blks = []
                for sb in range(n_sb):
                    blk = xtb_pool.tile([128, HC, SB], bf16, tag="xtb")
                    for c0 in range(0, HC, 4):
                        csl = slice(c0, c0 + 4)
                        nc.sync.dma_start(blk[:, csl, :],
                                          xtb_view[sb][:, csl, :])
                    xtb_blks.append(blk)
                for c0 in range(0, HC, 4):
                    csl = slice(c0, c0 + 4)
                    nc.scalar.dma_start(wv_t[:, csl, :], wv_view[:, csl, :])

                def emit_v_pass(sb):
                    xtb_blk = xtb_blks[sb]
                    for i in range(n_sb):
                        st = sb * 4 + i
                        isl = slice(i * 128, (i + 1) * 128)
                        ps = psA.tile([128, D_LOC], f32, tag="pqk", bufs=8,
                                      name=f"pv{sb}_{i}")
                        for hc in range(HC):
                            nc.tensor.matmul(
                                ps[:], xtb_blk[:, hc, isl], wv_t[:, hc, :],
                                start=(hc == 0), stop=(hc == HC - 1))
                        nc.scalar.copy(v_t[:, st, :], ps[:])

                for sb in range(n_sb):
                    ssl = slice(sb * SB, (sb + 1) * SB)
                    xt8_blk = xt8_blks[sb]
                    if sb >= 2:
                        # v-passes lag the q+k passes by one S-block: the
                        # bulk xtb/wv loads get an extra 14us to land, and
                        # each sb window has 2x PE work per RoPE batch so
                        # the DVE/GpSimd RoPE pipeline never gates the PE
                        emit_v_pass(sb - 2)

                    # interleaved q+k pass: both projections consume each
                    # xt8 pair-chunk as it lands (2x PE work per DMA byte
                    # keeps the PE fed during the cold-start window).  All
                    # 8 PSUM banks hold the 8 accumulators; k is allocated
                    # (and RoPE'd) first so its banks free up for the
                    # v-pass, whose first matmul then only waits on the
                    # first k-RoPE.  q's last pair goes after k's so the
                    # k RoPEs overlap q's trailing matmuls.
                    ps_k = [psA.tile([128, SB], f32, tag="pqk", bufs=8,
                                     name=f"pk{h}") for h in range(NH_LOC)]
                    ps_q = [psA.tile([128, SB], f32, tag="pqk", bufs=8,
                                     name=f"pq{h}") for h in range(NH_LOC)]

                    def qk_mm(ps_h, w_tile, c, h, start, stop):
                        cs2 = slice(2 * c, 2 * c + 2)
                        hs = slice(h * 128, (h + 1) * 128)
                        nc.tensor.matmul(
                            ps_h[h][:], w_tile[:, cs2, hs],
                            xt8_blk[:, cs2, :], start=start, stop=stop,
                            perf_mode=DR)

                    if sb == 0:
                        # pairs 0-1 chunk-wise (start on the first-landing
                        # DMA chunks), then finish each accumulator in turn
                        # so the RoPE pipeline overlaps the rest of the
                        # pass instead of gating sb1's PSUM banks.
                        for c in (0, 1):
                            for ps_h, w_tile in ((ps_k, wk_t), (ps_q, wq_t)):
                                for h in range(NH_LOC):
                                    qk_mm(ps_h, w_tile, c, h, c == 0, False)
                        for ps_h, w_tile in ((ps_k, wk_t), (ps_q, wq_t)):
                            for h in range(NH_LOC):
                                for c in range(2, NPAIR):
                                    qk_mm(ps_h, w_tile, c, h, False,
                                          c == NPAIR - 1)
                    else:
                        for c in range(NPAIR):
                            for ps_h, w_tile in ((ps_k, wk_t), (ps_q, wq_t)):
                                if c == NPAIR - 1 and w_tile is wq_t:
                                    continue
                                for h in range(NH_LOC):
                                    qk_mm(ps_h, w_tile, c, h, c == 0,
                                          c == NPAIR - 1)
                        for h in range(NH_LOC):
                            qk_mm(ps_q, wq_t, NPAIR - 1, h, False, True)
                    for ps_h, dst in ((ps_k, kT), (ps_q, qT)):
                        for h in range(NH_LOC):
                            # RoPE: dst = ps*cos + shuffle(ps)*sin_pm
                            # (fp8 descale + softmax scale are folded into
                            # the exp activation's scale, not the tables).
                            # Split across DVE (shuffle + cos-mul, the two
                            # PSUM reads) and GpSimd (sin-mul + add) so the
                            # RoPE pipeline keeps up with PSUM-bank reuse.
                            ps = ps_h[h]
                            t_sw = rope_pool.tile([128, SB], f32, tag="sw")
                            nc.vector.stream_shuffle(t_sw[:], ps[:], _SHUF_MASK)
                            t_cs = rope_pool.tile([128, SB], bf16, tag="cs")
                            nc.vector.tensor_mul(t_cs[:], ps[:], cos_t[:, ssl])
                            t_pr = rope_pool.tile([128, SB], bf16, tag="pr")
                            nc.gpsimd.tensor_mul(t_pr[:], t_sw[:], sin_t[:, ssl])
                            nc.gpsimd.tensor_add(dst[:, h, ssl], t_cs[:], t_pr[:])

                emit_v_pass(2)
                emit_v_pass(3)

            # ------------- Phase B: attention, Phase C: out proj -------------
            with (
                tc.tile_pool(name="wo", bufs=1) as wo_pool,
                tc.tile_pool(name="ot", bufs=1) as ot_pool,
            ):
                wo_t = wo_pool.tile([128, NH_LOC, H], bf16, tag="wo")
                nc.sync.dma_start(
                    wo_t[:], wot_d[:, :].rearrange("(dc p) o -> p dc o", p=128))
                ot_t = ot_pool.tile([128, NH_LOC, S], bf16, tag="ot")

                with (
                    tc.tile_pool(name="pt", bufs=4) as pt_pool,
                    tc.tile_pool(name="scr", bufs=2) as scr_pool,
                    tc.tile_pool(name="rcp", bufs=2) as rcp_pool,
                    tc.tile_pool(name="stage", bufs=6) as stage_pool,
                    tc.tile_pool(name="psB", bufs=1, space="PSUM") as psB,
                ):
                    # phase-C work units (st, hb), emitted interleaved with
                    # phase B so the in-order PE has filler during exp waits
                    c_units = []
                    out_qs = [nc.sync, nc.gpsimd]
                    out_qi = [0]

                    def emit_c_unit(use_scalar=False):
                        st, hb = c_units.pop(0)
                        stsl = slice(st * 128, (st + 1) * 128)
                        ps_c = psB.tile([128, SB], f32, tag="pc", bufs=2,
                                        name=f"pc_{st}_{hb}")
                        for dc in range(NH_LOC):
                            nc.tensor.matmul(
                                ps_c[:],
                                ot_t[:, dc, stsl],
                                wo_t[:, dc, hb * SB:(hb + 1) * SB],
                                start=(dc == 0), stop=(dc == NH_LOC - 1))
                        o_sb = stage_pool.tile([128, SB], bf16, tag="st",
                                               bufs=6)
                        if use_scalar:
                            nc.scalar.copy(o_sb[:], ps_c[:])
                        else:
                            nc.vector.tensor_copy(o_sb[:], ps_c[:])
                        q = out_qs[out_qi[0] % len(out_qs)]
                        out_qi[0] += 1
                        q.dma_start(
                            out_d[stsl, hb * SB:(hb + 1) * SB], o_sb[:])

                    tri = tri_t[:, :]  # keep f >= p triangle
                    # qb=0 (4 score tiles/head) has the worst exp-latency
                    # exposure and no phase-C filler if processed first;
                    # run qb=1 first so qb=0 can interleave its out-proj
                    # units.  Each qb is self-contained at this point.
                    for qb in (1, 0, 2, 3):
                        qsl = slice(qb * SB, (qb + 1) * SB)
                        nkt = 4 * (qb + 1)
                        for h in range(NH_LOC):
                            hs = slice(h * 128, (h + 1) * 128)
                            last = (qb, h) == (3, 3)
                            blk = pt_pool.tile([128, 16, SB], bf16, tag="pt")
                            ps_o = psB.tile([128, SB], f32, tag="o", bufs=2)
                            for c0 in range(0, nkt, 4):
                                for kt in range(c0, c0 + 4):
                                    j = kt - 4 * qb
                                    off = 128 * j if j > 0 else 0
                                    W = SB - off
                                    ksl = slice(kt * 128, (kt + 1) * 128)
                                    ps_s = psB.tile(
                                        [128, SB], f32, tag="s", bufs=3)
                                    nc.tensor.matmul(
                                        ps_s[:, 0:W], kT[:, h, ksl],
                                        qT[:, h, qb * SB + off:(qb + 1) * SB],
                                        start=True, stop=True)
                                    nc.scalar.activation(
                                        blk[:, kt, off:SB], ps_s[:, 0:W], Exp,
                                        scale=EXP_SCALE)
                                    if j >= 0:
                                        nc.vector.tensor_mul(
                                            blk[:, kt, off:off + 128],
                                            blk[:, kt, off:off + 128], tri)
                                for kt in range(c0, c0 + 4):
                                    j = kt - 4 * qb
                                    off = 128 * j if j > 0 else 0
                                    nc.tensor.matmul(
                                        ps_o[:, off:SB], v_t[:, kt, hs],
                                        blk[:, kt, off:SB],
                                        start=(kt == 0),
                                        stop=(kt == nkt - 1))
                                # phase-C filler for the PE during exp waits;
                                # reserve most units for the ACT-bound qb=3,
                                # and most of those for h=3 whose exp tail
                                # gates the final drain
                                n_fill = (2 if qb == 0 else 1) if qb < 3 \
                                    else (1 if h < 2 else (2 if h == 2 else 4))
                                for _ in range(n_fill):
                                    if c_units:
                                        emit_c_unit()

                            if last:
                                # last iteration: PE ones-matmul sums; a DVE
                                # tree here would sit exposed on the tail.
                                # Diagonal tiles are only partially written,
                                # so fold them into diag0 (full-width) first.
                                d0 = nkt - 4
                                for j in range(1, 4):
                                    o = 128 * j
                                    nc.vector.tensor_add(
                                        blk[:, d0, o:SB], blk[:, d0, o:SB],
                                        blk[:, d0 + j, o:SB])
                                ps_sum = psB.tile([128, SB], f32, tag="sum",
                                                  bufs=1)
                                for kt in range(d0 + 1):
                                    nc.tensor.matmul(
                                        ps_sum[:], ones_t[:], blk[:, kt, :],
                                        start=(kt == 0), stop=(kt == d0))
                                rcp = rcp_pool.tile([128, SB], f32, tag="rcp")
                                nc.vector.reciprocal_approx_fast(
                                    rcp[:], ps_sum[:])
                                nc.vector.tensor_mul(
                                    ot_t[:, h, qsl], ps_o[:], rcp[:])
                                continue
                            # sums: elementwise kt-tree on DVE (bf16), then
                            # one all-ones matmul reduces partitions+broadcasts
                            scr = scr_pool.tile([128, 12, SB], bf16, tag="scr")
                            nd = nkt - 4  # non-diagonal count
                            # fold diag j=1..3 into diag j=0 (valid suffixes)
                            d0 = nkt - 4 + 0
                            for j in range(1, 4):
                                o = 128 * j
                                nc.vector.tensor_add(
                                    blk[:, d0, o:SB], blk[:, d0, o:SB],
                                    blk[:, d0 + j, o:SB])
                            if nd == 0:
                                sums_src = blk[:, d0, :]
                            else:
                                # pairwise-halve the nd non-diag tiles
                                nc.vector.tensor_add(
                                    scr[:, 0:nd // 2, :],
                                    blk[:, 0:nd:2, :], blk[:, 1:nd:2, :])
                                m = nd // 2
                                base = 0
                                while m > 1:
                                    nb = base + m
                                    nc.vector.tensor_add(
                                        scr[:, nb:nb + m // 2, :],
                                        scr[:, base:base + m - 1:2, :],
                                        scr[:, base + 1:base + m:2, :])
                                    if m % 2:
                                        # carry odd leftover
                                        nc.vector.tensor_add(
                                            scr[:, nb, :], scr[:, nb, :],
                                            scr[:, base + m - 1, :])
                                    base, m = nb, m // 2
                                nc.vector.tensor_add(
                                    scr[:, base, :], scr[:, base, :],
                                    blk[:, d0, :])
                                sums_src = scr[:, base, :]
                            ps_sum = psB.tile([128, SB], f32, tag="sum",
                                              bufs=1)
                            nc.tensor.matmul(ps_sum[:], ones_t[:],
                                             sums_src, start=True, stop=True)
                            rcp = rcp_pool.tile([128, SB], f32, tag="rcp")
                            nc.vector.reciprocal_approx_fast(rcp[:], ps_sum[:])
                            nc.vector.tensor_mul(
                                ot_t[:, h, qsl], ps_o[:], rcp[:])
                        # this qb's output rows are now fully available
                        for st in range(qb * 4, qb * 4 + 4):
                            for hb in range(4):
                                c_units.append((st, hb))
                    # final drain: scalar is idle here, so add its queue
                    # to the out-DMA rotation to shorten the tail
                    out_qs.append(nc.scalar)
                    drain_i = 0
                    while c_units:
                        emit_c_unit(use_scalar=(drain_i % 2 == 0))
                        drain_i += 1

    nc.compile()
    return nc


def _host_prep(x, freq_cos, freq_sin, wq, wk, wv, wo):
    """Build the 8 per-core input maps."""
    cos_p = freq_cos.T[_J_OF_P, :].astype(np.float32)
    sin_p = (freq_sin.T[_J_OF_P, :] * _SIGN_OF_P[:, None]).astype(np.float32)
    cos_pm = np.ascontiguousarray(cos_p).astype(_BF16)
    sin_pm = np.ascontiguousarray(sin_p).astype(_BF16)

    f = np.arange(128)[None, :]
    p = np.arange(128)[:, None]
    tri = (f - p >= 0).astype(_BF16)
    ones = np.ones((128, 128), dtype=_BF16)

    def _repack_x(xt):
        # [H, S] -> [sb, p, hc, f] chunk-contiguous per partition row
        r = xt.reshape(HC, 128, 4, SB).transpose(2, 1, 0, 3)
        return np.ascontiguousarray(r).reshape(4 * 128, HC * SB)

    def _repack_w(wt):
        # [H, D_LOC] -> [p, hc, d] chunk-contiguous per partition row
        r = wt.reshape(HC, 128, D_LOC).transpose(1, 0, 2)
        return np.ascontiguousarray(r).reshape(128, HC * D_LOC)

    xt8_b = [_repack_x((x[b].T * SX8).astype(_F8)) for b in range(2)]
    xtb_b = [_repack_x(x[b].T.astype(_BF16)) for b in range(2)]

    in_maps = []
    for c in range(N_CORES):
        b, g = c // 4, c % 4
        rows = slice(g * D_LOC, (g + 1) * D_LOC)
        wq_g = wq[rows, :].reshape(NH_LOC, HD, H)[:, _P_NEW2OLD, :]
        wk_g = wk[rows, :].reshape(NH_LOC, HD, H)[:, _P_NEW2OLD, :]
        in_maps.append({
            "xt8": xt8_b[b],
            "xtb": xtb_b[b],
            "wq8": _repack_w(
                (wq_g.reshape(D_LOC, H).T * SW8).astype(_F8)),
            "wk8": _repack_w(
                (wk_g.reshape(D_LOC, H).T * SW8).astype(_F8)),
            "wvt": _repack_w(wv[rows, :].T.astype(_BF16)),
            "wot": np.ascontiguousarray(wo[:, rows].T).astype(_BF16),
            "cos_pm": cos_pm, "sin_pm": sin_pm,
            "tri": tri,
            "ones": ones,
        })
    return in_maps


def _kernel_np_fallback(x, freq_cos, freq_sin, attention_mask, wq, wk, wv, wo):
    """Numpy fallback (only used if attention_mask is nonzero)."""
    B = x.shape[0]
    hd = H // NHEADS
    q = (x @ wq.T).reshape(B, S, NHEADS, hd)
    k = (x @ wk.T).reshape(B, S, NHEADS, hd)
    v = (x @ wv.T).reshape(B, S, NHEADS, hd)

    def rope(t):
        x1, x2 = t[..., ::2], t[..., 1::2]
        c = freq_cos[None, :, None, :]
        s = freq_sin[None, :, None, :]
        o = np.empty_like(t)
        o[..., ::2] = x1 * c - x2 * s
        o[..., 1::2] = x1 * s + x2 * c
        return o

    q, k = rope(q), rope(k)
    q = q.transpose(0, 2, 1, 3)
    k = k.transpose(0, 2, 1, 3)
    v = v.transpose(0, 2, 1, 3)
    att = np.einsum("bhqd,bhkd->bhqk", q, k) / np.sqrt(hd) / (LAYER_INDEX + 1)
    att = att + attention_mask
    causal = np.triu(np.full((S, S), -1e30, dtype=att.dtype), k=1)
    att = att + causal[None, None]
    att = att - att.max(axis=-1, keepdims=True)
    att = np.exp(att)
    att = att / att.sum(axis=-1, keepdims=True)
    out = np.einsum("bhqk,bhkd->bhqd", att, v)
    out = out.transpose(0, 2, 1, 3).reshape(B, S, H)
    return (out @ wo.T).astype(np.float32)


def kernel(x, freq_cos, freq_sin, attention_mask, wq, wk, wv, wo, **extra):
    x = np.asarray(x, dtype=np.float32)
    freq_cos = np.asarray(freq_cos, dtype=np.float32)
    freq_sin = np.asarray(freq_sin, dtype=np.float32)
    attention_mask = np.asarray(attention_mask, dtype=np.float32)
    wq = np.asarray(wq, dtype=np.float32)
    wk = np.asarray(wk, dtype=np.float32)
    wv = np.asarray(wv, dtype=np.float32)
    wo = np.asarray(wo, dtype=np.float32)

    if attention_mask.any():
        # the device kernel folds the (all-zero) pad mask away
        return _kernel_np_fallback(
            x, freq_cos, freq_sin, attention_mask, wq, wk, wv, wo)

    from concourse.bass_utils import run_bass_kernel_spmd

    if "nc" not in _NC_CACHE:
        _NC_CACHE["nc"] = _build_nc()
    nc = _NC_CACHE["nc"]

    in_maps = _host_prep(x, freq_cos, freq_sin, wq, wk, wv, wo)
    res = run_bass_kernel_spmd(nc, in_maps, list(range(N_CORES)))

    out = np.zeros((2, S, H), dtype=np.float32)
    for c in range(N_CORES):
        out[c // 4] += res.results[c]["out_partial"].astype(np.float32)
    return out


# revision 30
# speedup vs baseline: 1.0109x; 1.0109x over previous
"""Trainium2 Bass kernel for causal multi-head attention (8-core SPMD).

Problem: B=2, S=2048, H=2048, 16 heads (hd=128), RoPE, causal mask,
layer-index scaling (/4), additive pad mask (zeros by construction).

Sharding: core c handles batch b=c//4 and head-group g=c%4 (4 heads).
wq/wk/wv column-parallel, wo row-parallel; host sums the 4 partial
outputs per batch.

Per-core dataflow (all feature-on-partition, "transposed" layouts):
  qT/kT [d=128, S] = w.T-tile @ xT      (PSUM accum over H-chunks)
    Q/K projections run in fp8 e4m3 with DoubleRow perf mode (2 128-row
    contraction chunks per matmul, 2x PE throughput).  x and w are
    pre-scaled into fp8 range on host; the descale and the softmax
    scale are folded into the exp activation's scale argument.
  RoPE on qT/kT via head-dim permutation chosen so the rotation pair
    sits 16 partitions apart inside each 32-partition quadrant
    (stream_shuffle does the swap in one DVE op)
  scoresT [k,q] tiles = kT-tile.T @ qT-block ; exp on ACT -> PT (bf16)
  diagonal tiles masked multiplicatively post-exp
  row sums via all-ones stationary matmul (broadcast over partitions)
  OT [d, q] += v-tile.T @ PT ; normalized by reciprocal(sums)
  out_partial = OT.T @ woT  (PSUM accum over the 4 local head chunks)

V projection, scores, attV and the out projection run in bf16 (fp32
PSUM accumulation); softmax math in fp32.  Output partials ship bf16.
"""
import math
import os
import sys

import numpy as np

for _p in ("/opt/trn_rl_repo", "/root/.axon_site/_ro/trn_rl_repo"):
    if os.path.isdir(_p) and _p not in sys.path:
        sys.path.append(_p)

import ml_dtypes

S = 2048
H = 2048
NHEADS = 16
HD = 128
NH_LOC = 4          # heads per core
D_LOC = NH_LOC * HD  # 512
LAYER_INDEX = 3
SCALE = 1.0 / (math.sqrt(HD) * (LAYER_INDEX + 1))
N_CORES = 8
SB = 512            # S-block (matmul moving free dim)
HC = H // 128       # contraction chunks
NPAIR = HC // 2     # DoubleRow pair-chunks

# fp8 pre-scales: keep quantized values in e4m3's normal range
SW8 = 8.0           # weight scale (w sigma 0.022 -> 0.18)
SX8 = 1.0           # x scale (already unit variance)
DESCALE = 1.0 / (SW8 * SX8)

# head-dim permutation: RoPE pair (x1_j, x2_j) -> rows (qd*32 + j%16,
# qd*32 + 16 + j%16) with qd = j//16, so the swap is within-quadrant.
_P_NEW2OLD = np.zeros(HD, dtype=np.int64)
_J_OF_P = np.zeros(HD, dtype=np.int64)
_SIGN_OF_P = np.zeros(HD, dtype=np.float32)
for _p in range(HD):
    _qd, _r = _p // 32, _p % 32
    _j = _qd * 16 + (_r % 16)
    _P_NEW2OLD[_p] = 2 * _j + (1 if _r >= 16 else 0)
    _J_OF_P[_p] = _j
    _SIGN_OF_P[_p] = 1.0 if _r >= 16 else -1.0
_SHUF_MASK = [(i + 16) % 32 for i in range(32)]

_BF16 = ml_dtypes.bfloat16
_F8 = ml_dtypes.float8_e4m3
_NC_CACHE = {}


def _build_nc():
    import concourse.bacc as bacc
    import concourse.mybir as mybir
    import concourse.tile as tile

    f32 = mybir.dt.float32
    bf16 = mybir.dt.bfloat16
    f8 = mybir.dt.float8e4
    DR = mybir.MatmulPerfMode.DoubleRow
    Exp = mybir.ActivationFunctionType.Exp

    nc = bacc.Bacc("TRN2", target_bir_lowering=False, debug=False)

    # x and weight layouts are host-repacked "[p-outer, chunk-contiguous]"
    # so every DMA line is 2-4KB (512B lines throttle early bandwidth)
    xt8_d = nc.dram_tensor("xt8", [4 * 128, HC * SB], f8,
                           kind="ExternalInput")
    xtb_d = nc.dram_tensor("xtb", [4 * 128, HC * SB], bf16,
                           kind="ExternalInput")
    wq8_d = nc.dram_tensor("wq8", [128, HC * D_LOC], f8,
                           kind="ExternalInput")
    wk8_d = nc.dram_tensor("wk8", [128, HC * D_LOC], f8,
                           kind="ExternalInput")
    wvt_d = nc.dram_tensor("wvt", [128, HC * D_LOC], bf16,
                           kind="ExternalInput")
    wot_d = nc.dram_tensor("wot", [D_LOC, H], bf16, kind="ExternalInput")
    cos_d = nc.dram_tensor("cos_pm", [128, S], bf16, kind="ExternalInput")
    sin_d = nc.dram_tensor("sin_pm", [128, S], bf16, kind="ExternalInput")
    tri_d = nc.dram_tensor("tri", [128, 128], bf16, kind="ExternalInput")
    ones_d = nc.dram_tensor("ones", [128, 128], bf16, kind="ExternalInput")
    out_d = nc.dram_tensor("out_partial", [S, H], bf16, kind="ExternalOutput")

    n_sb = S // SB       # 4
    n_st = S // 128      # 16
    EXP_SCALE = SCALE * DESCALE * DESCALE

    with tile.TileContext(nc) as tc:
        with (
            tc.tile_pool(name="const", bufs=1) as const_pool,
            tc.tile_pool(name="qkv", bufs=1) as qkv_pool,
        ):
            cos_t = const_pool.tile([128, S], bf16, tag="cos")
            sin_t = const_pool.tile([128, S], bf16, tag="sin")
            tri_t = const_pool.tile([128, 128], bf16, tag="tri")
            ones_t = const_pool.tile([128, 128], bf16, tag="ones")

            qT = qkv_pool.tile([128, NH_LOC, S], bf16, tag="qT")
            kT = qkv_pool.tile([128, NH_LOC, S], bf16, tag="kT")
            v_t = qkv_pool.tile([128, n_st, D_LOC], bf16, tag="v")

            # ---------------- Phase A: projections + RoPE ----------------
            with (
                tc.tile_pool(name="w", bufs=1) as w_pool,
                tc.tile_pool(name="xtp8", bufs=2) as xt8_pool,
                tc.tile_pool(name="xtpb", bufs=4) as xtb_pool,
                tc.tile_pool(name="rope", bufs=2) as rope_pool,
                tc.tile_pool(name="psA", bufs=2, space="PSUM") as psA,
            ):
                wq_t = w_pool.tile([128, HC, D_LOC], f8, tag="wq")
                wk_t = w_pool.tile([128, HC, D_LOC], f8, tag="wk")
                wv_t = w_pool.tile([128, HC, D_LOC], bf16, tag="wv")

                # PE warmup: the HAM clock gate needs ~3.4us of sustained
                # matmul activity to lift the cold 1.2GHz throttle, and the
                # first DMA chunks only land ~9us in.  Run throwaway
                # matmuls on a memset tile so the real projections start
                # at full clock.
                warm_sb = rope_pool.tile([128, SB], bf16, tag="warm", bufs=1)
                nc.vector.memset(warm_sb[:], 0)
                warm_ps = psA.tile([128, SB], f32, tag="pqk", bufs=8,
                                   name="warm")
                for i in range(12):
                    nc.tensor.matmul(warm_ps[:], warm_sb[:, 0:128],
                                     warm_sb[:], start=(i == 0),
                                     stop=(i == 11))

                xt8_view = xt8_d[:, :].rearrange(
                    "(sb p) (hc f) -> sb p hc f", p=128, hc=HC)
                xtb_view = xtb_d[:, :].rearrange(
                    "(sb p) (hc f) -> sb p hc f", p=128, hc=HC)
                wq_view = wq8_d[:, :].rearrange("p (hc d) -> p hc d", hc=HC)
                wk_view = wk8_d[:, :].rearrange("p (hc d) -> p hc d", hc=HC)
                wv_view = wvt_d[:, :].rearrange("p (hc d) -> p hc d", hc=HC)

                # Early-DMA priority: the interleaved q+k passes need xt8 +
                # wq8 + wk8 chunks immediately — one critical stream per
                # queue so none is starved.  RoPE tables follow (needed at
                # ~+20us), the bulk xtb/wv loads last (v-passes run after
                # all q+k, ~+60us).
                chunks = [(0, 2), (2, 2), (4, 4), (8, 4), (12, 4)]
                xt8_blks = []
                for sb in range(n_sb):
                    blk = xt8_pool.tile([128, HC, SB], f8, tag="xt8",
                                        bufs=3)
                    for c0, w in (chunks if sb == 0 else [(0, 8), (8, 8)]):
                        csl = slice(c0, c0 + w)
                        nc.sync.dma_start(blk[:, csl, :],
                                          xt8_view[sb][:, csl, :])
                    xt8_blks.append(blk)
                for c0, w in chunks:
                    csl = slice(c0, c0 + w)
                    nc.scalar.dma_start(wq_t[:, csl, :], wq_view[:, csl, :])
                    nc.gpsimd.dma_start(wk_t[:, csl, :], wk_view[:, csl, :])
                nc.gpsimd.dma_start(sin_t[:], sin_d[:, :])
                nc.gpsimd.dma_start(cos_t[:], cos_d[:, :])
                nc.gpsimd.dma_start(tri_t[:], tri_d[:, :])
                nc.gpsimd.dma_start(ones_t[:], ones_d[:, :])
                xtb_blks = []
                for sb in range(n_sb):
                    blk = xtb_pool.tile([128, HC, SB], bf16, tag="xtb")
                    for c0 in range(0, HC, 4):
                        csl = slice(c0, c0 + 4)
                        nc.sync.dma_start(blk[:, csl, :],
                                          xtb_view[sb][:, csl, :])
                    xtb_blks.append(blk)
                for c0 in range(0, HC, 4):
                    csl = slice(c0, c0 + 4)
                    nc.scalar.dma_start(wv_t[:, csl, :], wv_view[:, csl, :])

                def emit_v_pass(sb):
                    xtb_blk = xtb_blks[sb]
                    for i in range(n_sb):
                        st = sb * 4 + i
                        isl = slice(i * 128, (i + 1) * 128)
                        ps = psA.tile([128, D_LOC], f32, tag="pqk", bufs=8,
                                      name=f"pv{sb}_{i}")
                        for hc in range(HC):
                            nc.tensor.matmul(
                                ps[:], xtb_blk[:, hc, isl], wv_t[:, hc, :],
                                start=(hc == 0), stop=(hc == HC - 1))
                        nc.scalar.copy(v_t[:, st, :], ps[:])

                for sb in range(n_sb):
                    ssl = slice(sb * SB, (sb + 1) * SB)
                    xt8_blk = xt8_blks[sb]
                    if sb >= 2:
                        # v-passes lag the q+k passes by one S-block: the
                        # bulk xtb/wv loads get an extra 14us to land, and
                        # each sb window has 2x PE work per RoPE batch so
                        # the DVE/GpSimd RoPE pipeline never gates the PE
                        emit_v_pass(sb - 2)

                    # interleaved q+k pass: both projections consume each
                    # xt8 pair-chunk as it lands (2x PE work per DMA byte
                    # keeps the PE fed during the cold-start window).  All
                    # 8 PSUM banks hold the 8 accumulators; k is allocated
                    # (and RoPE'd) first so its banks free up for the
                    # v-pass, whose first matmul then only waits on the
                    # first k-RoPE.  q's last pair goes after k's so the
                    # k RoPEs overlap q's trailing matmuls.
                    ps_k = [psA.tile([128, SB], f32, tag="pqk", bufs=8,
                                     name=f"pk{h}") for h in range(NH_LOC)]
                    ps_q = [psA.tile([128, SB], f32, tag="pqk", bufs=8,
                                     name=f"pq{h}") for h in range(NH_LOC)]

                    def qk_mm(ps_h, w_tile, c, h, start, stop):
                        cs2 = slice(2 * c, 2 * c + 2)
                        hs = slice(h * 128, (h + 1) * 128)
                        nc.tensor.matmul(
                            ps_h[h][:], w_tile[:, cs2, hs],
                            xt8_blk[:, cs2, :], start=start, stop=stop,
                            perf_mode=DR)

                    if sb == 0:
                        # pairs 0-1 chunk-wise (start on the first-landing
                        # DMA chunks), then finish each accumulator in turn
                        # so the RoPE pipeline overlaps the rest of the
                        # pass instead of gating sb1's PSUM banks.
                        for c in (0, 1):
                            for ps_h, w_tile in ((ps_k, wk_t), (ps_q, wq_t)):
                                for h in range(NH_LOC):
                                    qk_mm(ps_h, w_tile, c, h, c == 0, False)
                        for ps_h, w_tile in ((ps_k, wk_t), (ps_q, wq_t)):
                            for h in range(NH_LOC):
                                for c in range(2, NPAIR):
                                    qk_mm(ps_h, w_tile, c, h, False,
                                          c == NPAIR - 1)
                    else:
                        for c in range(NPAIR):
                            for ps_h, w_tile in ((ps_k, wk_t), (ps_q, wq_t)):
                                if c == NPAIR - 1 and w_tile is wq_t:
                                    continue
                                for h in range(NH_LOC):
                                    qk_mm(ps_h, w_tile, c, h, c == 0,
                                          c == NPAIR - 1)
                        for h in range(NH_LOC):
                            qk_mm(ps_q, wq_t, NPAIR - 1, h, False, True)
                    for ps_h, dst in ((ps_k, kT), (ps_q, qT)):
                        for h in range(NH_LOC):
                            # RoPE: dst = ps*cos + shuffle(ps)*sin_pm
                            # (fp8 descale + softmax scale are folded into
                            # the exp activation's scale, not the tables).
                            # Split across DVE (shuffle + cos-mul, the two
                            # PSUM reads) and GpSimd (sin-mul + add) so the
                            # RoPE pipeline keeps up with PSUM-bank reuse.
                            ps = ps_h[h]
                            t_sw = rope_pool.tile([128, SB], f32, tag="sw")
                            nc.vector.stream_shuffle(t_sw[:], ps[:], _SHUF_MASK)
                            t_cs = rope_pool.tile([128, SB], bf16, tag="cs")
                            nc.vector.tensor_mul(t_cs[:], ps[:], cos_t[:, ssl])
                            t_pr = rope_pool.tile([128, SB], bf16, tag="pr")
                            nc.gpsimd.tensor_mul(t_pr[:], t_sw[:], sin_t[:, ssl])
                            nc.gpsimd.tensor_add(dst[:, h, ssl], t_cs[:], t_pr[:])

                emit_v_pass(2)
                emit_v_pass(3)

            # ------------- Phase B: attention, Phase C: out proj -------------
            with (
                tc.tile_pool(name="wo", bufs=1) as wo_pool,
                tc.tile_pool(name="ot", bufs=1) as ot_pool,
            ):
                wo_t = wo_pool.tile([128, NH_LOC, H], bf16, tag="wo")
                nc.sync.dma_start(
                    wo_t[:], wot_d[:, :].rearrange("(dc p) o -> p dc o", p=128))
                ot_t = ot_pool.tile([128, NH_LOC, S], bf16, tag="ot")

                with (
                    tc.tile_pool(name="pt", bufs=4) as pt_pool,
                    tc.tile_pool(name="scr", bufs=2) as scr_pool,
                    tc.tile_pool(name="rcp", bufs=2) as rcp_pool,
                    tc.tile_pool(name="stage", bufs=6) as stage_pool,
                    tc.tile_pool(name="psB", bufs=1, space="PSUM") as psB,
                ):
                    # phase-C work units (st, hb), emitted interleaved with
                    # phase B so the in-order PE has filler during exp waits
                    c_units = []
                    out_qs = [nc.sync, nc.gpsimd]
                    out_qi = [0]

                    def emit_c_unit(use_scalar=False):
                        st, hb = c_units.pop(0)
                        stsl = slice(st * 128, (st + 1) * 128)
                        ps_c = psB.tile([128, SB], f32, tag="pc", bufs=2,
                                        name=f"pc_{st}_{hb}")
                        for dc in range(NH_LOC):
                            nc.tensor.matmul(
                                ps_c[:],
                                ot_t[:, dc, stsl],
                                wo_t[:, dc, hb * SB:(hb + 1) * SB],
                                start=(dc == 0), stop=(dc == NH_LOC - 1))
                        o_sb = stage_pool.tile([128, SB], bf16, tag="st",
                                               bufs=6)
                        if use_scalar:
                            nc.scalar.copy(o_sb[:], ps_c[:])
                        else:
                            nc.vector.tensor_copy(o_sb[:], ps_c[:])
                        q = out_qs[out_qi[0] % len(out_qs)]
                        out_qi[0] += 1
                        q.dma_start(
                            out_d[stsl, hb * SB:(hb + 1) * SB], o_sb[:])

                    tri = tri_t[:, :]  # keep f >= p triangle
                    # qb=0 (4 score tiles/head) has the worst exp-latency
                    # exposure and no phase-C filler if processed first;
                    # run qb=1 first so qb=0 can interleave its out-proj
                    # units.  Each qb is self-contained at this point.
                    for qb in (1, 0, 2, 3):
                        qsl = slice(qb * SB, (qb + 1) * SB)
                        nkt = 4 * (qb + 1)
                        for h in range(NH_LOC):
                            hs = slice(h * 128, (h + 1) * 128)
                            last = (qb, h) == (3, 3)
                            blk = pt_pool.tile([128, 16, SB], bf16, tag="pt")
                            ps_o = psB.tile([128, SB], f32, tag="o", bufs=2)
                            for c0 in range(0, nkt, 4):
                                for kt in range(c0, c0 + 4):
                                    j = kt - 4 * qb
                                    off = 128 * j if j > 0 else 0
                                    W = SB - off
                                    ksl = slice(kt * 128, (kt + 1) * 128)
                                    ps_s = psB.tile(
                                        [128, SB], f32, tag="s", bufs=3)
                                    nc.tensor.matmul(
                                        ps_s[:, 0:W], kT[:, h, ksl],
                                        qT[:, h, qb * SB + off:(qb + 1) * SB],
                                        start=True, stop=True)
                                    nc.scalar.activation(
                                        blk[:, kt, off:SB], ps_s[:, 0:W], Exp,
                                        scale=EXP_SCALE)
                                    if j >= 0:
                                        nc.vector.tensor_mul(
                                            blk[:, kt, off:off + 128],
                                            blk[:, kt, off:off + 128], tri)
                                for kt in range(c0, c0 + 4):
                                    j = kt - 4 * qb
                                    off = 128 * j if j > 0 else 0
                                    nc.tensor.matmul(
                                        ps_o[:, off:SB], v_t[:, kt, hs],
                                        blk[:, kt, off:SB],
                                        start=(kt == 0),
                                        stop=(kt == nkt - 1))
                                # phase-C filler for the PE during exp waits;
                                # reserve most units for the ACT-bound qb=3,
                                # and most of those for h=3 whose exp tail
                                # gates the final drain
                                n_fill = (2 if qb == 0 else 1) if qb < 3 \
                                    else (1 if h < 2 else (2 if h == 2 else 4))
                                for _ in range(n_fill):
                                    if c_units:
                                        emit_c_unit()

                            if last:
                                # last iteration: PE ones-matmul sums; a DVE
                                # tree here would sit exposed on the tail.
                                # Diagonal tiles are only partially written,
                                # so fold them into diag0 (full-width) first.
                                d0 = nkt - 4
                                for j in range(1, 4):
                                    o = 128 * j
                                    nc.vector.tensor_add(
                                        blk[:, d0, o:SB], blk[:, d0, o:SB],
                                        blk[:, d0 + j, o:SB])
                                ps_sum = psB.tile([128, SB], f32, tag="sum",
                                                  bufs=1)
                                for kt in range(d0 + 1):
                                    nc.tensor.matmul(
                                        ps_sum[:], ones_t[:], blk[:, kt, :],
                                        start=(kt == 0), stop=(kt == d0))
                                rcp = rcp_pool.tile([128, SB], f32, tag="rcp")
                                nc.vector.reciprocal_approx_fast(
                                    rcp[:], ps_sum[:])
                                nc.vector.tensor_mul(
                                    ot_t[:, h, qsl], ps_o[:], rcp[:])
                                continue
                            # sums: elementwise kt-tree on DVE (bf16), then
                            # one all-ones matmul reduces partitions+broadcasts
                            scr = scr_pool.tile([128, 12, SB], bf16, tag="scr")
                            nd = nkt - 4  # non-diagonal count
                            # fold diag j=1..3 into diag j=0 (valid suffixes)
                            d0 = nkt - 4 + 0
                            for j in range(1, 4):
                                o = 128 * j
                                nc.vector.tensor_add(
                                    blk[:, d0, o:SB], blk[:, d0, o:SB],
                                    blk[:, d0 + j, o:SB])
                            if nd == 0:
                                sums_src = blk[:, d0, :]
                            else:
                                # pairwise-halve the nd non-diag tiles
                                nc.vector.tensor_add(
                                    scr[:, 0:nd // 2, :],
                                    blk[:, 0:nd:2, :], blk[:, 1:nd:2, :])
                                m = nd // 2
                                base = 0
                                while m > 1:
                                    nb = base + m
                                    nc.vector.tensor_add(
                                        scr[:, nb:nb + m // 2, :],
                                        scr[:, base:base + m - 1:2, :],
                                        scr[:, base + 1:base + m:2, :])
                                    if m % 2:
                                        # carry odd leftover
                                        nc.vector.tensor_add(
                                            scr[:, nb, :], scr[:, nb, :],
                                            scr[:, base + m - 1, :])
                                    base, m = nb, m // 2
                                nc.vector.tensor_add(
                                    scr[:, base, :], scr[:, base, :],
                                    blk[:, d0, :])
                                sums_src = scr[:, base, :]
                            ps_sum = psB.tile([128, SB], f32, tag="sum",
                                              bufs=1)
                            nc.tensor.matmul(ps_sum[:], ones_t[:],
                                             sums_src, start=True, stop=True)
                            rcp = rcp_pool.tile([128, SB], f32, tag="rcp")
                            nc.vector.reciprocal_approx_fast(rcp[:], ps_sum[:])
                            nc.vector.tensor_mul(
                                ot_t[:, h, qsl], ps_o[:], rcp[:])
                        # this qb's output rows are now fully available
                        for st in range(qb * 4, qb * 4 + 4):
                            for hb in range(4):
                                c_units.append((st, hb))
                    # final drain: scalar is idle here, so add its queue
                    # to the out-DMA rotation to shorten the tail
                    out_qs.append(nc.scalar)
                    drain_i = 0
                    while c_units:
                        emit_c_unit(use_scalar=(drain_i % 2 == 0))
                        drain_i += 1

    nc.compile()
    return nc


def _host_prep(x, freq_cos, freq_sin, wq, wk, wv, wo):
    """Build the 8 per-core input maps."""
    cos_p = freq_cos.T[_J_OF_P, :].astype(np.float32)
    sin_p = (freq_sin.T[_J_OF_P, :] * _SIGN_OF_P[:, None]).astype(np.float32)
    cos_pm = np.ascontiguousarray(cos_p).astype(_BF16)
    sin_pm = np.ascontiguousarray(sin_p).astype(_BF16)

    f = np.arange(128)[None, :]
    p = np.arange(128)[:, None]
    tri = (f - p >= 0).astype(_BF16)
    ones = np.ones((128, 128), dtype=_BF16)

    def _repack_x(xt):
        # [H, S] -> [sb, p, hc, f] chunk-contiguous per partition row
        r = xt.reshape(HC, 128, 4, SB).transpose(2, 1, 0, 3)
        return np.ascontiguousarray(r).reshape(4 * 128, HC * SB)

    def _repack_w(wt):
        # [H, D_LOC] -> [p, hc, d] chunk-contiguous per partition row
        r = wt.reshape(HC, 128, D_LOC).transpose(1, 0, 2)
        return np.ascontiguousarray(r).reshape(128, HC * D_LOC)

    xt8_b = [_repack_x((x[b].T * SX8).astype(_F8)) for b in range(2)]
    xtb_b = [_repack_x(x[b].T.astype(_BF16)) for b in range(2)]

    in_maps = []
    for c in range(N_CORES):
        b, g = c // 4, c % 4
        rows = slice(g * D_LOC, (g + 1) * D_LOC)
        wq_g = wq[rows, :].reshape(NH_LOC, HD, H)[:, _P_NEW2OLD, :]
        wk_g = wk[rows, :].reshape(NH_LOC, HD, H)[:, _P_NEW2OLD, :]
        in_maps.append({
            "xt8": xt8_b[b],
            "xtb": xtb_b[b],
            "wq8": _repack_w(
                (wq_g.reshape(D_LOC, H).T * SW8).astype(_F8)),
            "wk8": _repack_w(
                (wk_g.reshape(D_LOC, H).T * SW8).astype(_F8)),
            "wvt": _repack_w(wv[rows, :].T.astype(_BF16)),
            "wot": np.ascontiguousarray(wo[:, rows].T).astype(_BF16),
            "cos_pm": cos_pm, "sin_pm": sin_pm,
            "tri": tri,
            "ones": ones,
        })
    return in_maps


def _kernel_np_fallback(x, freq_cos, freq_sin, attention_mask, wq, wk, wv, wo):
    """Numpy fallback (only used if attention_mask is nonzero)."""
    B = x.shape[0]
    hd = H // NHEADS
    q = (x @ wq.T).reshape(B, S, NHEADS, hd)
    k = (x @ wk.T).reshape(B, S, NHEADS, hd)
    v = (x @ wv.T).reshape(B, S, NHEADS, hd)

    def rope(t):
        x1, x2 = t[..., ::2], t[..., 1::2]
        c = freq_cos[None, :, None, :]
        s = freq_sin[None, :, None, :]
        o = np.empty_like(t)
        o[..., ::2] = x1 * c - x2 * s
        o[..., 1::2] = x1 * s + x2 * c
        return o

    q, k = rope(q), rope(k)
    q = q.transpose(0, 2, 1, 3)
    k = k.transpose(0, 2, 1, 3)
    v = v.transpose(0, 2, 1, 3)
    att = np.einsum("bhqd,bhkd->bhqk", q, k) / np.sqrt(hd) / (LAYER_INDEX + 1)
    att = att + attention_mask
    causal = np.triu(np.full((S, S), -1e30, dtype=att.dtype), k=1)
    att = att + causal[None, None]
    att = att - att.max(axis=-1, keepdims=True)
    att = np.exp(att)
    att = att / att.sum(axis=-1, keepdims=True)
    out = np.einsum("bhqk,bhkd->bhqd", att, v)
    out = out.transpose(0, 2, 1, 3).reshape(B, S, H)
    return (out @ wo.T).astype(np.float32)


def kernel(x, freq_cos, freq_sin, attention_mask, wq, wk, wv, wo, **extra):
    x = np.asarray(x, dtype=np.float32)
    freq_cos = np.asarray(freq_cos, dtype=np.float32)
    freq_sin = np.asarray(freq_sin, dtype=np.float32)
    attention_mask = np.asarray(attention_mask, dtype=np.float32)
    wq = np.asarray(wq, dtype=np.float32)
    wk = np.asarray(wk, dtype=np.float32)
    wv = np.asarray(wv, dtype=np.float32)
    wo = np.asarray(wo, dtype=np.float32)

    if attention_mask.any():
        # the device kernel folds the (all-zero) pad mask away
        return _kernel_np_fallback(
            x, freq_cos, freq_sin, attention_mask, wq, wk, wv, wo)

    from concourse.bass_utils import run_bass_kernel_spmd

    if "nc" not in _NC_CACHE:
        _NC_CACHE["nc"] = _build_nc()
    nc = _NC_CACHE["nc"]

    in_maps = _host_prep(x, freq_cos, freq_sin, wq, wk, wv, wo)
    res = run_bass_kernel_spmd(nc, in_maps, list(range(N_CORES)))

    out = np.zeros((2, S, H), dtype=np.float32)
    for c in range(N_CORES):
        out[c // 4] += res.results[c]["out_partial"].astype(np.float32)
    return out
